# revision 15
# baseline (speedup 1.0000x reference)
"""Trainium2 Bass kernel for segment_reduce (Raw2Alpha + Alphas2Weights).

Math (per sample i with ray r, interval=iv):
    lp     = -softplus(density + shift) * iv          # log(1 - alpha)
    alpha  = -expm1(lp)                               # = sigmoid(density+shift) when iv == 1
    T      = exp(exclusive within-ray cumsum of lp)   # transmittance
    weights          = T * alpha
    alphainv_last[r] = exp(sum of lp over ray r)      # prod(1-alpha) per ray

Device strategy (pure data parallel over rays, 8 cores):
  * Rays are distributed 8192 per core; each ray gets a fixed-width slot of
    L columns (L > max ray length) in a [128, F] layout, F = 64*L
    (64 ray slots per partition row).  Host pads density with -30.0
    (=> 1-alpha == 1.0 exactly in f32) so padded samples are multiplicative
    identities.
  * One ACT pass computes explp = 1-alpha = sigmoid(-(x+shift)); a second
    ACT pass (same sigmoid table set) computes alpha = sigmoid(x+shift)
    at full relative precision.
  * The per-ray exclusive cumprod T is ONE DVE tensor_tensor_scan per tile:
        state = data0*state + data1
    with data0 = explp shifted right one column and zeroed at slot starts,
    data1 = 1.0 at slot starts (constant tile).  This both restarts each ray
    at T=1 and yields the exclusive product.
  * alphainv_last per ray = scan value at its slot's last column (a pad
    column, so the running product covers the whole ray): a strided slice,
    no gather needed.
"""

import math
import os

import numpy as np

import concourse.bacc as bacc
import concourse.bass as bass
import concourse.mybir as mybir
import concourse.tile as tile
from concourse.bass_utils import run_bass_kernel_spmd

F32 = mybir.dt.float32
N_CORES = 8
P = 128  # partitions
PAD_VAL = -30.0  # sigmoid(-(PAD_VAL+shift)) rounds to exactly 1.0f for |shift|<10

_nc_cache = {}


def _build_nc(L, spr, spc, shift, interval):
    """Build the Bass program.

    L: slot width (columns per ray), spr: slots per partition row,
    spc: slots per chunk, shift/interval: baked scalar constants.
    """
    F = spr * L
    CH = spc * L
    nchunk = spr // spc
    assert spr % spc == 0

    nc = bacc.Bacc(
        "TRN2", target_bir_lowering=False, debug=False, num_devices=N_CORES
    )
    den = nc.dram_tensor("density", [P, F], F32, kind="ExternalInput")
    wout = nc.dram_tensor("weights", [P, F], F32, kind="ExternalOutput")
    aiout = nc.dram_tensor("alphainv", [P, spr], F32, kind="ExternalOutput")

    sig = mybir.ActivationFunctionType.Sigmoid
    use_sigmoid = interval == 1.0

    with tile.TileContext(nc) as tc:
        with (
            tc.tile_pool(name="io", bufs=4) as io,
            tc.tile_pool(name="work", bufs=3) as work,
            tc.tile_pool(name="const", bufs=1) as const,
        ):
            # data1 of the scan: 1.0 at slot-start columns, 0 elsewhere
            d1 = const.tile([P, CH], F32)
            nc.vector.memset(d1[:], 0.0)
            nc.vector.memset(d1[:, 0:CH:L], 1.0)
            ai = const.tile([P, spr], F32)
            # Warm up the sigmoid table set with a dependency-free op so the
            # ACT_TABLE_LOAD wait doesn't ride on a real (already-waiting)
            # activation and overflow its sync-wait slots.
            warm = const.tile([P, 1], F32)
            nc.scalar.activation(warm[:], warm[:], sig, bias=0.0, scale=1.0)
            # Bias constants built on the ACT engine itself (same-engine deps
            # need no semaphore waits): Copy computes in*scale + bias.
            copy_fn = mybir.ActivationFunctionType.Copy
            bpos = const.tile([P, 1], F32)
            nc.scalar.activation(bpos[:], warm[:], copy_fn, bias=shift, scale=0.0)
            bneg = const.tile([P, 1], F32)
            nc.scalar.activation(bneg[:], warm[:], copy_fn, bias=-shift, scale=0.0)

            for c in range(nchunk):
                sl = slice(c * CH, (c + 1) * CH)
                d = io.tile([P, CH], F32, tag="d")
                nc.sync.dma_start(d[:], den[:, sl])

                # ez[:, 1:CH+1] = explp = 1 - alpha; ez[:, j] holds sample j-1
                ez = work.tile([P, CH + 1], F32, tag="ez")
                if use_sigmoid:
                    nc.scalar.activation(
                        ez[:, 1 : CH + 1], d[:], sig, bias=bneg[:], scale=-1.0
                    )
                else:
                    sp = work.tile([P, CH], F32, tag="sp")
                    nc.scalar.activation(
                        sp[:],
                        d[:],
                        mybir.ActivationFunctionType.Softplus,
                        bias=bpos[:],
                        scale=1.0,
                    )
                    nc.scalar.activation(
                        ez[:, 1 : CH + 1],
                        sp[:],
                        mybir.ActivationFunctionType.Exp,
                        bias=0.0,
                        scale=-interval,
                    )
                # zero the shifted slot-start columns (these land on pad
                # samples, slot-local column L-1, so no data is lost)
                nc.vector.memset(ez[:, 0 : CH + 1 : L], 0.0)

                # alpha at full precision
                al = work.tile([P, CH], F32, tag="al")
                if use_sigmoid:
                    nc.scalar.activation(al[:], d[:], sig, bias=bpos[:], scale=1.0)
                else:
                    nc.vector.tensor_scalar(
                        al[:],
                        ez[:, 1 : CH + 1],
                        -1.0,
                        1.0,
                        mybir.AluOpType.mult,
                        mybir.AluOpType.add,
                    )

                # exclusive within-ray cumprod: state = ez*state + d1
                t = work.tile([P, CH], F32, tag="t")
                nc.vector.tensor_tensor_scan(
                    t[:],
                    ez[:, 0:CH],
                    d1[:],
                    0.0,
                    mybir.AluOpType.mult,
                    mybir.AluOpType.add,
                )

                # weights = alpha * T, split DVE/GPSIMD to balance engine load
                w = io.tile([P, CH], F32, tag="w")
                cut = (CH * 3 // 16) // 4 * 4
                nc.vector.tensor_tensor(
                    w[:, 0:cut], al[:, 0:cut], t[:, 0:cut], mybir.AluOpType.mult
                )
                nc.gpsimd.tensor_tensor(
                    w[:, cut:CH], al[:, cut:CH], t[:, cut:CH], mybir.AluOpType.mult
                )
                nc.sync.dma_start(wout[:, sl], w[:])

                # per-ray totals sit at slot-end columns
                nc.vector.tensor_copy(
                    ai[:, c * spc : (c + 1) * spc], t[:, L - 1 : CH : L]
                )

            nc.sync.dma_start(aiout[:], ai[:])
    nc.compile()
    return nc


def kernel(density, shift, interval, ray_id, n_rays, _bench=None):
    density = np.asarray(density, dtype=np.float32).reshape(-1)
    ray_id = np.asarray(ray_id).astype(np.int64).reshape(-1)
    shift_f = float(np.asarray(shift))
    interval_f = float(np.asarray(interval))
    n_rays = int(n_rays)
    M = density.shape[0]

    # ---- host-side layout (derived only from ray_id) ----
    rays_per_core = math.ceil(n_rays / N_CORES)
    spr = math.ceil(rays_per_core / P)  # slots (rays) per partition row
    spr = max(16, math.ceil(spr / 16) * 16)  # keep divisible by chunking
    rays_padded = N_CORES * P * spr
    rays_per_core_p = P * spr

    counts = np.bincount(ray_id, minlength=rays_padded)
    maxlen = int(counts.max()) if M else 1
    L = ((maxlen + 2) + 3) // 4 * 4  # slot width, multiple of 4, >= maxlen+1
    F = spr * L
    spc = 8  # slots per chunk
    key = (L, spr, spc, shift_f, interval_f)
    if key not in _nc_cache:
        _nc_cache[key] = _build_nc(L, spr, spc, shift_f, interval_f)
    nc = _nc_cache[key]

    starts = np.zeros(rays_padded + 1, dtype=np.int64)
    np.cumsum(counts, out=starts[1:])
    off = np.arange(M, dtype=np.int64) - starts[ray_id]
    row = ray_id // spr  # global row index 0..8*128-1
    sc = ray_id % spr
    pos = row * F + sc * L + off  # position in the concatenated padded array

    padded = np.full(N_CORES * P * F, PAD_VAL, dtype=np.float32)
    padded[pos] = density
    padded = padded.reshape(N_CORES, P, F)

    in_maps = [{"density": padded[c]} for c in range(N_CORES)]
    res = run_bass_kernel_spmd(
        nc,
        in_maps,
        core_ids=list(range(N_CORES)),
        trace=bool(os.environ.get("BASS_TRACE")),
    )
    if _bench is not None:
        _bench.append(res)

    w_all = np.stack([res.results[c]["weights"] for c in range(N_CORES)])
    weights = np.ascontiguousarray(w_all.reshape(-1)[pos])
    ai_all = np.stack([res.results[c]["alphainv"] for c in range(N_CORES)])
    alphainv = np.ascontiguousarray(ai_all.reshape(-1)[:n_rays])
    return weights, alphainv


# revision 16
# speedup vs baseline: 1.0772x; 1.0772x over previous
"""Trainium2 Bass kernel for segment_reduce (Raw2Alpha + Alphas2Weights).

Math (per sample i with ray r, interval=iv):
    lp     = -softplus(density + shift) * iv          # log(1 - alpha)
    alpha  = -expm1(lp)                               # = sigmoid(density+shift) when iv == 1
    T      = exp(exclusive within-ray cumsum of lp)   # transmittance
    weights          = T * alpha
    alphainv_last[r] = exp(sum of lp over ray r)      # prod(1-alpha) per ray

Device strategy (pure data parallel over rays, 8 cores):
  * Rays are distributed 8192 per core; each ray gets a fixed-width slot of
    L columns (L > max ray length) in a [128, F] layout, F = 64*L
    (64 ray slots per partition row).  Host pads density with -30.0
    (=> 1-alpha == 1.0 exactly in f32) so padded samples are multiplicative
    identities.
  * One ACT pass computes explp = 1-alpha = sigmoid(-(x+shift)); a second
    ACT pass (same sigmoid table set) computes alpha = sigmoid(x+shift)
    at full relative precision.
  * The per-ray exclusive cumprod T is ONE DVE tensor_tensor_scan per tile:
        state = data0*state + data1
    with data0 = explp shifted right one column and zeroed at slot starts,
    data1 = 1.0 at slot starts (constant tile).  This both restarts each ray
    at T=1 and yields the exclusive product.
  * alphainv_last per ray = scan value at its slot's last column (a pad
    column, so the running product covers the whole ray): a strided slice,
    no gather needed.
"""

import math
import os

import numpy as np

import concourse.bacc as bacc
import concourse.bass as bass
import concourse.mybir as mybir
import concourse.tile as tile
from concourse.bass_utils import run_bass_kernel_spmd

F32 = mybir.dt.float32
N_CORES = 8
P = 128  # partitions
PAD_VAL = -30.0  # sigmoid(-(PAD_VAL+shift)) rounds to exactly 1.0f for |shift|<10

_nc_cache = {}


def _build_nc(L, spr, spc, shift, interval):
    """Build the Bass program.

    L: slot width (columns per ray), spr: slots per partition row,
    spc: slots per chunk, shift/interval: baked scalar constants.
    """
    F = spr * L
    CH = spc * L
    nchunk = spr // spc
    assert spr % spc == 0

    nc = bacc.Bacc(
        "TRN2", target_bir_lowering=False, debug=False, num_devices=N_CORES
    )
    den = nc.dram_tensor("density", [P, F], F32, kind="ExternalInput")
    wout = nc.dram_tensor("weights", [P, F], F32, kind="ExternalOutput")
    aiout = nc.dram_tensor("alphainv", [P, spr], F32, kind="ExternalOutput")

    sig = mybir.ActivationFunctionType.Sigmoid
    use_sigmoid = interval == 1.0

    with tile.TileContext(nc) as tc:
        with (
            tc.tile_pool(name="io", bufs=4) as io,
            tc.tile_pool(name="work", bufs=3) as work,
            tc.tile_pool(name="const", bufs=1) as const,
        ):
            # data1 of the scan: 1.0 at slot-start columns, 0 elsewhere
            d1 = const.tile([P, CH], F32)
            nc.vector.memset(d1[:], 0.0)
            nc.vector.memset(d1[:, 0:CH:L], 1.0)
            ai = const.tile([P, spr], F32)
            # Warm up the sigmoid table set with a dependency-free op so the
            # ACT_TABLE_LOAD wait doesn't ride on a real (already-waiting)
            # activation and overflow its sync-wait slots.
            warm = const.tile([P, 1], F32)
            nc.scalar.activation(warm[:], warm[:], sig, bias=0.0, scale=1.0)
            # Bias constants built on the ACT engine itself (same-engine deps
            # need no semaphore waits): Copy computes in*scale + bias.
            copy_fn = mybir.ActivationFunctionType.Copy
            bpos = const.tile([P, 1], F32)
            nc.scalar.activation(bpos[:], warm[:], copy_fn, bias=shift, scale=0.0)
            bneg = const.tile([P, 1], F32)
            nc.scalar.activation(bneg[:], warm[:], copy_fn, bias=-shift, scale=0.0)

            for c in range(nchunk):
                sl = slice(c * CH, (c + 1) * CH)
                d = io.tile([P, CH], F32, tag="d")
                nc.sync.dma_start(d[:], den[:, sl])

                # ez[:, 1:CH+1] = explp = 1 - alpha; ez[:, j] holds sample j-1
                ez = work.tile([P, CH + 1], F32, tag="ez")
                if use_sigmoid:
                    nc.scalar.activation(
                        ez[:, 1 : CH + 1], d[:], sig, bias=bneg[:], scale=-1.0
                    )
                else:
                    sp = work.tile([P, CH], F32, tag="sp")
                    nc.scalar.activation(
                        sp[:],
                        d[:],
                        mybir.ActivationFunctionType.Softplus,
                        bias=bpos[:],
                        scale=1.0,
                    )
                    nc.scalar.activation(
                        ez[:, 1 : CH + 1],
                        sp[:],
                        mybir.ActivationFunctionType.Exp,
                        bias=0.0,
                        scale=-interval,
                    )
                # zero the shifted slot-start columns (these land on pad
                # samples, slot-local column L-1, so no data is lost)
                nc.vector.memset(ez[:, 0 : CH + 1 : L], 0.0)

                # alpha at full precision
                al = work.tile([P, CH], F32, tag="al")
                if use_sigmoid:
                    nc.scalar.activation(al[:], d[:], sig, bias=bpos[:], scale=1.0)
                else:
                    nc.vector.tensor_scalar(
                        al[:],
                        ez[:, 1 : CH + 1],
                        -1.0,
                        1.0,
                        mybir.AluOpType.mult,
                        mybir.AluOpType.add,
                    )

                # exclusive within-ray cumprod: state = ez*state + d1
                t = work.tile([P, CH], F32, tag="t")
                nc.vector.tensor_tensor_scan(
                    t[:],
                    ez[:, 0:CH],
                    d1[:],
                    0.0,
                    mybir.AluOpType.mult,
                    mybir.AluOpType.add,
                )

                # weights = alpha * T, split DVE/GPSIMD to balance engine load
                w = io.tile([P, CH], F32, tag="w")
                cut = (CH * 3 // 16) // 4 * 4
                nc.vector.tensor_tensor(
                    w[:, 0:cut], al[:, 0:cut], t[:, 0:cut], mybir.AluOpType.mult
                )
                nc.gpsimd.tensor_tensor(
                    w[:, cut:CH], al[:, cut:CH], t[:, cut:CH], mybir.AluOpType.mult
                )
                nc.sync.dma_start(wout[:, sl], w[:])

                # per-ray totals sit at slot-end columns (ACT is least busy)
                nc.scalar.copy(ai[:, c * spc : (c + 1) * spc], t[:, L - 1 : CH : L])

            nc.sync.dma_start(aiout[:], ai[:])
    nc.compile()
    return nc


def kernel(density, shift, interval, ray_id, n_rays, _bench=None):
    density = np.asarray(density, dtype=np.float32).reshape(-1)
    ray_id = np.asarray(ray_id).astype(np.int64).reshape(-1)
    shift_f = float(np.asarray(shift))
    interval_f = float(np.asarray(interval))
    n_rays = int(n_rays)
    M = density.shape[0]

    # ---- host-side layout (derived only from ray_id) ----
    rays_per_core = math.ceil(n_rays / N_CORES)
    spr = math.ceil(rays_per_core / P)  # slots (rays) per partition row
    spr = max(16, math.ceil(spr / 16) * 16)  # keep divisible by chunking
    rays_padded = N_CORES * P * spr
    rays_per_core_p = P * spr

    counts = np.bincount(ray_id, minlength=rays_padded)
    maxlen = int(counts.max()) if M else 1
    L = ((maxlen + 2) + 3) // 4 * 4  # slot width, multiple of 4, >= maxlen+1
    F = spr * L
    spc = 8  # slots per chunk
    key = (L, spr, spc, shift_f, interval_f)
    if key not in _nc_cache:
        _nc_cache[key] = _build_nc(L, spr, spc, shift_f, interval_f)
    nc = _nc_cache[key]

    starts = np.zeros(rays_padded + 1, dtype=np.int64)
    np.cumsum(counts, out=starts[1:])
    off = np.arange(M, dtype=np.int64) - starts[ray_id]
    row = ray_id // spr  # global row index 0..8*128-1
    sc = ray_id % spr
    pos = row * F + sc * L + off  # position in the concatenated padded array

    padded = np.full(N_CORES * P * F, PAD_VAL, dtype=np.float32)
    padded[pos] = density
    padded = padded.reshape(N_CORES, P, F)

    in_maps = [{"density": padded[c]} for c in range(N_CORES)]
    res = run_bass_kernel_spmd(
        nc,
        in_maps,
        core_ids=list(range(N_CORES)),
        trace=bool(os.environ.get("BASS_TRACE")),
    )
    if _bench is not None:
        _bench.append(res)

    w_all = np.stack([res.results[c]["weights"] for c in range(N_CORES)])
    weights = np.ascontiguousarray(w_all.reshape(-1)[pos])
    ai_all = np.stack([res.results[c]["alphainv"] for c in range(N_CORES)])
    alphainv = np.ascontiguousarray(ai_all.reshape(-1)[:n_rays])
    return weights, alphainv


# revision 17
# speedup vs baseline: 1.3599x; 1.2625x over previous
"""Trainium2 Bass kernel for segment_reduce (Raw2Alpha + Alphas2Weights).

Math (per sample i of ray r, interval = 1):
    explp  = 1 - alpha = sigmoid(-(density + shift))      # = exp(log(1-alpha))
    alpha  = sigmoid(density + shift)
    T      = exclusive within-ray cumprod of explp        # transmittance
    weights          = T * alpha
    alphainv_last[r] = prod of explp over ray r

Device strategy (pure data parallel over rays, 8 cores):
  * Host packs rays into fixed-width slots of L columns in a [128, F]
    layout (F = spr*L, spr slots per partition row).  Rays longer than
    L-1 samples are split across ADJACENT slots in the same row/chunk.
    Pads use density=-30 so explp == 1.0 exactly (multiplicative identity).
  * One DVE tensor_tensor_scan per tile computes the exclusive segmented
    cumprod:  state = data0*state + data1, with
       data0 = explp shifted right one column; at slot-start columns it
               holds the host "continuation mask" (0.0 = reset = new ray,
               1.0 = ray continues from previous slot);
       data1 = (1 - mask) at slot starts (re-seeds state to 1), 0 elsewhere.
    For continuation slots the running product simply flows through.
  * alpha via a second sigmoid (same ACT table set — no table thrash).
  * Per-ray totals land at slot-end columns: strided slice, no gather.
"""

import math
import os
from collections import deque

import numpy as np

import concourse.bacc as bacc
import concourse.mybir as mybir
import concourse.tile as tile
from concourse.bass_utils import run_bass_kernel_spmd

F32 = mybir.dt.float32
N_CORES = 8
P = 128  # partitions
PAD_VAL = -30.0  # sigmoid(-(PAD_VAL+shift)) rounds to exactly 1.0f for |shift|<10

_nc_cache = {}


def _build_nc(L, spr, spc, shift, interval):
    """Bass program: L slot width, spr slots/row, spc slots/chunk."""
    F = spr * L
    nchunk = math.ceil(spr / spc)

    nc = bacc.Bacc(
        "TRN2", target_bir_lowering=False, debug=False, num_devices=N_CORES
    )
    den = nc.dram_tensor("density", [P, F], F32, kind="ExternalInput")
    maskin = nc.dram_tensor("contmask", [P, spr], F32, kind="ExternalInput")
    wout = nc.dram_tensor("weights", [P, F], F32, kind="ExternalOutput")
    aiout = nc.dram_tensor("alphainv", [P, spr], F32, kind="ExternalOutput")

    sig = mybir.ActivationFunctionType.Sigmoid
    copy_fn = mybir.ActivationFunctionType.Copy
    use_sigmoid = interval == 1.0
    CHM = spc * L

    with tile.TileContext(nc) as tc:
        with (
            tc.tile_pool(name="io", bufs=4) as io,
            tc.tile_pool(name="work", bufs=3) as work,
            tc.tile_pool(name="const", bufs=1) as const,
        ):
            maskt = const.tile([P, spr], F32)
            nc.sync.dma_start(maskt[:], maskin[:])
            onem = const.tile([P, spr], F32)  # 1 - mask
            nc.vector.tensor_scalar(
                onem[:], maskt[:], -1.0, 1.0, mybir.AluOpType.mult, mybir.AluOpType.add
            )
            d1c = const.tile([P, CHM], F32)  # scan data1: zeros + seeds at starts
            nc.vector.memset(d1c[:], 0.0)
            ai = const.tile([P, spr], F32)
            # Warm up the sigmoid table set with a dependency-free op so the
            # ACT_TABLE_LOAD wait doesn't ride on a real activation.
            warm = const.tile([P, 1], F32)
            nc.scalar.activation(warm[:], warm[:], sig, bias=0.0, scale=1.0)
            # Bias constants built on the ACT engine (same-engine deps are free)
            bpos = const.tile([P, 1], F32)
            nc.scalar.activation(bpos[:], warm[:], copy_fn, bias=shift, scale=0.0)
            bneg = const.tile([P, 1], F32)
            nc.scalar.activation(bneg[:], warm[:], copy_fn, bias=-shift, scale=0.0)

            for c in range(nchunk):
                s0 = c * spc
                s1 = min(s0 + spc, spr)
                ns = s1 - s0
                w = ns * L
                sl = slice(s0 * L, s1 * L)
                d = io.tile([P, CHM], F32, tag="d")
                nc.sync.dma_start(d[:, :w], den[:, sl])

                # ez[:, j] holds explp of sample j-1 (shifted)
                ez = work.tile([P, CHM + 1], F32, tag="ez")
                if use_sigmoid:
                    nc.scalar.activation(
                        ez[:, 1 : w + 1], d[:, :w], sig, bias=bneg[:], scale=-1.0
                    )
                else:
                    sp = work.tile([P, CHM], F32, tag="sp")
                    nc.scalar.activation(
                        sp[:, :w],
                        d[:, :w],
                        mybir.ActivationFunctionType.Softplus,
                        bias=bpos[:],
                        scale=1.0,
                    )
                    nc.scalar.activation(
                        ez[:, 1 : w + 1],
                        sp[:, :w],
                        mybir.ActivationFunctionType.Exp,
                        bias=0.0,
                        scale=-interval,
                    )
                # slot-start columns: 0.0 resets a new ray, 1.0 continues one
                nc.vector.tensor_copy(ez[:, 0:w:L], maskt[:, s0:s1])
                # scan data1: 1-mask seeds state=1 at resets
                nc.vector.tensor_copy(d1c[:, 0:w:L], onem[:, s0:s1])

                # alpha at full precision
                al = work.tile([P, CHM], F32, tag="al")
                if use_sigmoid:
                    nc.scalar.activation(
                        al[:, :w], d[:, :w], sig, bias=bpos[:], scale=1.0
                    )
                else:
                    nc.vector.tensor_scalar(
                        al[:, :w],
                        ez[:, 1 : w + 1],
                        -1.0,
                        1.0,
                        mybir.AluOpType.mult,
                        mybir.AluOpType.add,
                    )

                # exclusive segmented cumprod: state = ez*state + d1c
                t = work.tile([P, CHM], F32, tag="t")
                nc.vector.tensor_tensor_scan(
                    t[:, :w],
                    ez[:, 0:w],
                    d1c[:, 0:w],
                    0.0,
                    mybir.AluOpType.mult,
                    mybir.AluOpType.add,
                )

                # weights = alpha * T
                wt = io.tile([P, CHM], F32, tag="w")
                nc.vector.tensor_tensor(
                    wt[:, :w], al[:, :w], t[:, :w], mybir.AluOpType.mult
                )
                nc.sync.dma_start(wout[:, sl], wt[:, :w])

                # per-ray running totals sit at slot-end columns
                nc.scalar.copy(ai[:, s0:s1], t[:, L - 1 : w : L])

            nc.sync.dma_start(aiout[:], ai[:])
    nc.compile()
    return nc


def _pack_core(nch, spr, spc):
    """Assign each ray's chunks to consecutive slots of a [128, spr] grid.

    nch: per-ray chunk counts (1 or 2).  Multi-chunk rays must occupy
    adjacent slots within one row and one spc-wide chunk block.
    Returns (row[r], slot0[r]) or None if it doesn't fit in 128 rows.
    """
    n = len(nch)
    row = np.empty(n, dtype=np.int32)
    slot0 = np.empty(n, dtype=np.int32)
    pend = deque(range(n))
    r_i, s_i = 0, 0
    while pend:
        if r_i >= P:
            return None
        ray = pend[0]
        c = nch[ray]
        ok = s_i + c <= spr and (c == 1 or (s_i + 1) % spc != 0)
        if ok:
            pend.popleft()
            row[ray] = r_i
            slot0[ray] = s_i
            s_i += c
        else:
            # find a later single-chunk ray to fill this slot
            filled = False
            for k in range(1, min(len(pend), 64)):
                alt = pend[k]
                if nch[alt] == 1:
                    del pend[k]
                    row[alt] = r_i
                    slot0[alt] = s_i
                    s_i += 1
                    filled = True
                    break
            if not filled:
                s_i += 1  # waste the slot
        if s_i >= spr:
            r_i += 1
            s_i = 0
    return row, slot0


def kernel(density, shift, interval, ray_id, n_rays, _bench=None):
    density = np.asarray(density, dtype=np.float32).reshape(-1)
    ray_id = np.asarray(ray_id).astype(np.int64).reshape(-1)
    shift_f = float(np.asarray(shift))
    interval_f = float(np.asarray(interval))
    n_rays = int(n_rays)
    M = density.shape[0]

    # ---- host-side layout (derived only from ray_id) ----
    rays_per_core = math.ceil(n_rays / N_CORES)
    rays_padded = N_CORES * rays_per_core

    counts = np.bincount(ray_id, minlength=rays_padded).astype(np.int64)
    maxlen = max(int(counts.max()), 1) if M else 1
    spc = 8

    # choose slot width L minimizing F = spr*L (pads vs splits trade-off)
    best = None
    for L in range(((maxlen + 4) // 4) * 4 + 4, 32, -4):
        cap = L - 1
        if maxlen > 2 * cap:
            break  # keep at most 2 chunks per ray
        worst = 0
        for c in range(N_CORES):
            cc = counts[c * rays_per_core : (c + 1) * rays_per_core]
            slots = int(np.maximum(1, -(-cc // cap)).sum())
            worst = max(worst, -(-slots // P))
        F = worst * L
        if best is None or F < best[0]:
            best = (F, L, worst)
    _, L, spr = best
    cap = L - 1

    # pack every core; grow spr on (rare) packing overflow
    packs = []
    while True:
        ok = True
        packs = []
        for c in range(N_CORES):
            cc = counts[c * rays_per_core : (c + 1) * rays_per_core]
            nch = np.maximum(1, -(-cc // cap)).astype(np.int32)
            got = _pack_core(nch, spr, spc)
            if got is None:
                ok = False
                break
            packs.append(got)
        if ok:
            break
        spr += 1
    F = spr * L

    key = (L, spr, spc, shift_f, interval_f)
    if key not in _nc_cache:
        _nc_cache[key] = _build_nc(L, spr, spc, shift_f, interval_f)
    nc = _nc_cache[key]

    # per-ray placement arrays (global)
    g_row = np.empty(rays_padded, dtype=np.int64)
    g_slot0 = np.empty(rays_padded, dtype=np.int64)
    for c in range(N_CORES):
        row, slot0 = packs[c]
        g_row[c * rays_per_core : (c + 1) * rays_per_core] = row + c * P
        g_slot0[c * rays_per_core : (c + 1) * rays_per_core] = slot0

    # per-sample positions in the concatenated [8*128, F] padded array
    starts = np.zeros(rays_padded + 1, dtype=np.int64)
    np.cumsum(counts, out=starts[1:])
    off = np.arange(M, dtype=np.int64) - starts[ray_id]
    pos = (
        g_row[ray_id] * F
        + (g_slot0[ray_id] + off // cap) * L
        + off % cap
    )

    padded = np.full(N_CORES * P * F, PAD_VAL, dtype=np.float32)
    padded[pos] = density
    padded = padded.reshape(N_CORES, P, F)

    # continuation mask: 1.0 at every non-first chunk slot of a ray
    contmask = np.zeros((N_CORES * P, spr), dtype=np.float32)
    multi = np.nonzero(counts > cap)[0]
    for r in multi:
        for j in range(1, int(-(-counts[r] // cap))):
            contmask[g_row[r], g_slot0[r] + j] = 1.0
    contmask = contmask.reshape(N_CORES, P, spr)

    in_maps = [
        {"density": padded[c], "contmask": contmask[c]} for c in range(N_CORES)
    ]
    res = run_bass_kernel_spmd(
        nc,
        in_maps,
        core_ids=list(range(N_CORES)),
        trace=bool(os.environ.get("BASS_TRACE")),
    )
    if _bench is not None:
        _bench.append(res)

    w_all = np.stack([res.results[c]["weights"] for c in range(N_CORES)])
    weights = np.ascontiguousarray(w_all.reshape(-1)[pos])
    # alphainv: running product at the ray's LAST chunk's slot end
    ai_all = np.stack([res.results[c]["alphainv"] for c in range(N_CORES)])
    nch_all = np.maximum(1, -(-counts // cap)).astype(np.int64)
    ai_pos = g_row * spr + g_slot0 + nch_all - 1
    alphainv = np.ascontiguousarray(ai_all.reshape(-1)[ai_pos][:n_rays])
    return weights, alphainv
